# revision 13
# baseline (speedup 1.0000x reference)
"""HGNNPConv Trainium2 kernel (8 NeuronCores, SPMD).

Math (equivalent reformulation of the reference):
  Xe_raw[e] = mean_{i: e_idx[i]=e} X[v_idx[i]]              (v2e, softmax of ones = 1/deg)
  Xe_p      = Xe_raw @ W.T + b                              (GEMM on 4000 edges, not 20000 verts)
  Xv[v]     = sum_i softmax_w[i] * Xe_p[e_idx[i]]           (e2v, weights host-normalized)
  out       = relu(Xv)
Empty edges get a spurious +b in Xe_p but are never referenced downstream.

Sharding: phase 1 by destination edge (500/core; 4 windows of 128), edge-level
GEMM per core, AllGather of the projected edge table (per-window when
CC_MODE=split, so collectives overlap later windows' phase-1 work), phase 2 by
destination vertex (2500/core; 20 windows).

Selection matrices are built on the HOST (one [128,128] block per gathered
chunk, shipped as a constant table): sources are DEDUPLICATED per window (each
distinct row gathered once; its sel column pattern carries all destinations and
summed weights), cutting gather indices ~12% and removing the per-chunk DVE
iota-compare chain entirely. Phase-1 sel is binary; the exact 1/deg scale is
applied per-window in f32. Phase-2 sel carries host-normalized softmax weights.
Chunk counts per window are ragged (host schedule baked into the build).
All tables bf16 (fp8 fails the 2e-2 gate); accumulation f32 PSUM.
"""

import os
from contextlib import ExitStack

import numpy as np
import ml_dtypes

# ---------------------------------------------------------------- config ---
NCORES = 8
NV, NE, NNZ, CH = 20000, 4000, 160000, 512
DT = os.environ.get("KERNEL_DT", "bf16")  # bf16 | fp8 (fp8 fails the 2e-2 gate)
GRP = int(os.environ.get("KERNEL_GRP", "8"))   # gather chunks (of 128 idxs) per call
GBUFS = int(os.environ.get("KERNEL_GBUFS", "3"))
NQUEUES = int(os.environ.get("KERNEL_NQ", "1"))
CC_MODE = os.environ.get("KERNEL_CC", "single")  # split | single | none
TRACE = os.environ.get("BASS_TRACE", "") != ""

_last_results = None   # BassKernelResults of the most recent run (for test.py)


# ------------------------------------------------------------------- plan ---
class Plan:
    pass


def _binpack(ids, degs, nbins, cap=128):
    """Pack `ids` into `nbins` bins of <=cap items, balancing sum(degs)."""
    import heapq

    order = np.argsort(-degs, kind="stable")
    bins = [[] for _ in range(nbins)]
    loads = [0] * nbins
    heap = [(0, b) for b in range(nbins)]
    heapq.heapify(heap)
    for t in order:
        popped = []
        while True:
            load, b = heapq.heappop(heap)
            if len(bins[b]) < cap:
                break
            popped.append((load, b))
        for p in popped:
            heapq.heappush(heap, p)
        bins[b].append(int(ids[t]))
        loads[b] = load + int(degs[t])
        heapq.heappush(heap, (loads[b], b))
    return bins, loads


def _csr(idx, n):
    order = np.argsort(idx, kind="stable").astype(np.int64)
    deg = np.bincount(idx, minlength=n).astype(np.int64)
    starts = np.zeros(n + 1, np.int64)
    np.cumsum(deg, out=starts[1:])
    return order, deg, starts


def _dedup_phase(bins_per_core, order, starts, idx_of_inc, w_of_inc):
    """Deduped per-core schedule for one aggregation phase.

    Returns (cw, gidx, sel, members): cw[w] = chunk count of window w;
    gidx[c*128+q] = source row of slot q in (global) chunk c;
    sel[q, c*128+j] = summed weight of incidences slot q -> dest j (chunk c's
    window); members[w][j] = destination id at window w row j.
    """
    cw = []
    gidx_parts = []
    sel_parts = []
    members = []
    for bin_ids in bins_per_core:
        srcs, dests, ws = [], [], []
        for j, d in enumerate(bin_ids):
            seg = order[starts[d]:starts[d + 1]]
            srcs.append(idx_of_inc[seg])
            dests.append(np.full(len(seg), j, np.int64))
            ws.append(w_of_inc[seg])
        srcs = np.concatenate(srcs) if srcs else np.zeros(0, np.int64)
        dests = np.concatenate(dests) if dests else np.zeros(0, np.int64)
        ws = (np.concatenate(ws) if ws else np.zeros(0, np.float64)).astype(np.float64)
        uniq, slot = np.unique(srcs, return_inverse=True)
        n = len(uniq)
        nck = max(1, -(-n // 128))
        cw.append(nck)
        g = np.zeros(nck * 128, np.int16)
        g[:n] = uniq
        s = np.zeros((128, nck * 128), np.float64)
        np.add.at(s, (slot % 128, (slot // 128) * 128 + dests), ws)
        gidx_parts.append(g)
        sel_parts.append(s)
        members.append(bin_ids)
    gidx = np.concatenate(gidx_parts)
    sel = np.concatenate(sel_parts, axis=1).astype(np.float32)
    return cw, gidx, sel, members


def _wrap_idx(flat):
    """int16 flat[i] -> [128, len/16] with value i at [i%16, i//16], replicated."""
    a = flat.reshape(-1, 16).T  # [16, L/16]
    return np.ascontiguousarray(np.tile(a, (8, 1)))


def make_plan(v_idx, e_idx, e2v_weight, nv=NV, ne=NE, ch=CH, ncores=NCORES,
              cc_mode=None):
    cc_mode = cc_mode or CC_MODE
    P = Plan()
    P.nv, P.ne, P.ch, P.ncores = nv, ne, ch, ncores
    epc, vpc = ne // ncores, nv // ncores

    v_idx = v_idx.astype(np.int64)
    e_idx = e_idx.astype(np.int64)
    order_e, deg_e, starts_e = _csr(e_idx, ne)
    order_v, deg_v, starts_v = _csr(v_idx, nv)
    inv_deg = np.zeros(ne, np.float32)
    nz = deg_e > 0
    inv_deg[nz] = np.float32(1.0) / deg_e[nz].astype(np.float32)

    # host-normalized e2v softmax weights (exact, f32)
    ew = np.exp(e2v_weight.astype(np.float64))
    den = np.zeros(nv, np.float64)
    np.add.at(den, v_idx, ew)
    wnorm = ew / np.where(den[v_idx] > 0, den[v_idx], 1.0)

    nb1 = -(-epc // 128)
    nb2 = -(-vpc // 128)
    P.NW1, P.NW2 = nb1, nb2
    bins1, bins2 = [], []
    for k in range(ncores):
        eids = np.arange(k * epc, (k + 1) * epc)
        bins1.append(_binpack(eids, deg_e[eids], nb1)[0])
        vids = np.arange(k * vpc, (k + 1) * vpc)
        bins2.append(_binpack(vids, deg_v[vids], nb2)[0])

    # phase 1: dedup by source vertex; binary sel; edge position map
    ones = np.ones(len(v_idx), np.float64)
    pos = np.zeros(ne, np.int64)
    P.p1 = []
    P.invd = []
    ner1 = P.NW1 * 128
    for k in range(ncores):
        cw, gidx, sel, members = _dedup_phase(
            bins1[k], order_e, starts_e, v_idx, ones)
        P.p1.append((cw, gidx, sel))
        invd = np.zeros((128, P.NW1), np.float32)
        for w, bin_ids in enumerate(members):
            for j, e in enumerate(bin_ids):
                if cc_mode == "split":
                    # CCOUT row for edge e: window-major allgathered layout
                    pos[e] = w * (128 * ncores) + k * 128 + j
                else:
                    # core-major layout of a single whole-table AllGather
                    pos[e] = k * ner1 + w * 128 + j
                invd[j, w] = inv_deg[e]
        P.invd.append(invd)
    assert pos.max() < 32768

    # phase 2: dedup by source edge position; sel carries summed softmax weights
    P.p2 = []
    P.vmap = []
    for k in range(ncores):
        cw, gidx, sel, members = _dedup_phase(
            bins2[k], order_v, starts_v, pos[e_idx], wnorm)
        P.p2.append((cw, gidx, sel))
        vm = np.full(P.NW2 * 128, -1, np.int64)
        for w, bin_ids in enumerate(members):
            vm[w * 128:w * 128 + len(bin_ids)] = bin_ids
        P.vmap.append(vm)

    # chunk schedules must be uniform across cores for one SPMD program:
    # pad each window's chunk count to the max over cores
    P.cw1 = [max(P.p1[k][0][w] for k in range(ncores)) for w in range(P.NW1)]
    P.cw2 = [max(P.p2[k][0][w] for k in range(ncores)) for w in range(P.NW2)]
    P.C1 = sum(P.cw1)
    P.C2 = sum(P.cw2)

    def _repad(phase, cws, C):
        out = []
        for (cw, gidx, sel) in phase:
            g = np.zeros(C * 128, np.int16)
            s = np.zeros((128, C * 128), np.float32)
            src_c = dst_c = 0
            for w, nck in enumerate(cws):
                have = cw[w]
                g[dst_c * 128:(dst_c + have) * 128] = gidx[src_c * 128:(src_c + have) * 128]
                s[:, dst_c * 128:(dst_c + have) * 128] = sel[:, src_c * 128:(src_c + have) * 128]
                src_c += have
                dst_c += nck
            out.append((g, s))
        return out

    P.p1 = _repad(P.p1, P.cw1, P.C1)
    P.p2 = _repad(P.p2, P.cw2, P.C2)
    return P


# ---------------------------------------------------------------- builder ---
def build_nc(P, dt=DT, spmd=True, reps=1, grp=GRP, gbufs=GBUFS, nqueues=NQUEUES,
             cc_mode=None):
    cc_mode = cc_mode or CC_MODE
    import concourse.bacc as bacc
    import concourse.mybir as mybir
    import concourse.tile as tile

    f32 = mybir.dt.float32
    dt_g = mybir.dt.float8e4 if dt == "fp8" else mybir.dt.bfloat16
    bf16 = mybir.dt.bfloat16
    mul, mx, add = mybir.AluOpType.mult, mybir.AluOpType.max, mybir.AluOpType.add
    ch, KT = P.ch, P.ch // 128

    nc = bacc.Bacc("TRN2", target_bir_lowering=False, debug=False,
                   num_devices=P.ncores if spmd else 1,
                   num_swdge_queues=nqueues)

    XT = nc.dram_tensor("xt", [P.nv, ch], dt_g, kind="ExternalInput")
    WT = nc.dram_tensor("wt", [128, KT, ch], bf16, kind="ExternalInput")
    BREP = nc.dram_tensor("brep", [128, ch], f32, kind="ExternalInput")
    IDENT = nc.dram_tensor("ident", [128, 128], f32, kind="ExternalInput")
    INVD = nc.dram_tensor("invd", [128, P.NW1], f32, kind="ExternalInput")
    G1IDX = nc.dram_tensor("g1idx", [128, P.C1 * 8], mybir.dt.int16, kind="ExternalInput")
    SEL1 = nc.dram_tensor("sel1", [128, P.C1 * 128], dt_g, kind="ExternalInput")
    G2IDX = nc.dram_tensor("g2idx", [128, P.C2 * 8], mybir.dt.int16, kind="ExternalInput")
    SEL2 = nc.dram_tensor("sel2", [128, P.C2 * 128], dt_g, kind="ExternalInput")

    ner1 = P.NW1 * 128
    CCIN = nc.dram_tensor("ccin", [ner1, ch], dt_g)
    CCOUT = nc.dram_tensor("ccout", [P.ncores * ner1, ch], dt_g, addr_space="Shared")
    OUT = nc.dram_tensor("out", [P.NW2 * 128, ch], f32, kind="ExternalOutput")

    with tile.TileContext(nc) as tc, ExitStack() as ctx:
        const = ctx.enter_context(tc.tile_pool(name="const", bufs=1))
        gpool = ctx.enter_context(tc.tile_pool(name="g", bufs=gbufs))
        psum = ctx.enter_context(tc.tile_pool(name="ps", bufs=2, space="PSUM"))
        sbp = ctx.enter_context(tc.tile_pool(name="sbp", bufs=2))
        xe_pool = ctx.enter_context(tc.tile_pool(name="xe", bufs=1))

        def cload(dram, shape, dtt, tag):
            t = const.tile(shape, dtt, tag=tag)
            nc.sync.dma_start(t[:], dram[:])
            return t

        wt_t = cload(WT, [128, KT, ch], bf16, "wt")
        brep_t = cload(BREP, [128, ch], f32, "brep")
        ident_t = cload(IDENT, [128, 128], f32, "ident")
        invd_t = cload(INVD, [128, P.NW1], f32, "invd")
        g1idx_t = cload(G1IDX, [128, P.C1 * 8], mybir.dt.int16, "g1idx")
        sel1_t = cload(SEL1, [128, P.C1 * 128], dt_g, "sel1")
        g2idx_t = cload(G2IDX, [128, P.C2 * 8], mybir.dt.int16, "g2idx")
        sel2_t = cload(SEL2, [128, P.C2 * 128], dt_g, "sel2")

        def agg_phase(src_ap, gidx_t, sel_t, cws, gtag, win_cb):
            """Gather chunks of 128 rows, reduce via shipped one-hot blocks."""
            C = sum(cws)
            # window/posn of each global chunk
            wofc, pofc = [], []
            for w, nck in enumerate(cws):
                wofc += [w] * nck
                pofc += list(range(nck))
            pw = None
            for g0 in range(0, C, grp):
                n = min(grp, C - g0)
                gt = gpool.tile([128, n, ch], dt_g, tag=gtag)
                nc.gpsimd.dma_gather(
                    gt[:], src_ap, gidx_t[:, g0 * 8:(g0 + n) * 8],
                    n * 128, n * 128, ch, queue_num=(g0 // grp) % nqueues)
                for j in range(n):
                    c = g0 + j
                    w, cwp = wofc[c], pofc[c]
                    last = (cwp == cws[w] - 1)
                    if cwp == 0:
                        pw = psum.tile([128, ch], f32, tag="win")
                    nc.tensor.matmul(pw[:], sel_t[:, c * 128:(c + 1) * 128],
                                     gt[:, j, :], start=(cwp == 0), stop=last)
                    if last:
                        win_cb(pw, w)

        state = {}

        def p1_win(pw, w):
            # exact 1/deg scale (f32) while copying PSUM -> SBUF
            nc.vector.tensor_scalar(state["xe"][:, w, :], pw[:],
                                    invd_t[:, w:w + 1], None, op0=mul)
            # transpose Xe window -> [c_in, e] for the GEMM
            for k in range(KT):
                pt = psum.tile([128, 128], f32, tag="aux", name="pt")
                nc.tensor.transpose(pt[:], state["xe"][:, w, k * 128:(k + 1) * 128],
                                    ident_t[:])
                nc.vector.tensor_copy(state["xeT"][:, k, w * 128:(w + 1) * 128], pt[:])
            # GEMM: Xe_p = Xe_raw @ W.T + b for this 128-edge window
            pg = psum.tile([128, ch], f32, tag="aux", name="pg")
            for k in range(KT):
                nc.tensor.matmul(pg[:], state["xeT"][:, k, w * 128:(w + 1) * 128],
                                 wt_t[:, k, :], start=(k == 0), stop=(k == KT - 1))
            xep = sbp.tile([128, ch], dt_g, tag="xep", name="xep")
            nc.vector.tensor_tensor(xep[:], pg[:], brep_t[:], op=add)
            nc.sync.dma_start(CCIN[w * 128:(w + 1) * 128, :], xep[:])
            # per-window AllGather: overlaps later windows' phase-1 work
            if cc_mode == "split" and state["spmd"]:
                nc.gpsimd.collective_compute(
                    "AllGather", mybir.AluOpType.bypass,
                    replica_groups=[list(range(P.ncores))],
                    ins=[CCIN[w * 128:(w + 1) * 128, :]],
                    outs=[CCOUT[w * 128 * P.ncores:(w + 1) * 128 * P.ncores, :]])
            elif cc_mode != "single" or not state["spmd"]:
                # local-write stand-in (cost-model build / timing probe)
                off = (w * 128 * P.ncores) if cc_mode == "split" else (w * 128)
                nc.sync.dma_start(CCOUT[off:off + 128, :], xep[:])

        def p2_win(pw, w):
            ow = sbp.tile([128, ch], f32, tag="ow", name="ow")
            nc.vector.tensor_scalar(ow[:], pw[:], 1.0, 0.0, op0=mul, op1=mx)
            nc.sync.dma_start(OUT[w * 128:(w + 1) * 128, :], ow[:])

        state["spmd"] = spmd
        for _rep in range(reps):
            xe_t = xe_pool.tile([128, P.NW1, ch], f32, tag="xe", name="xe")
            xeT_t = xe_pool.tile([128, KT, ner1], bf16, tag="xeT", name="xeT")
            state["xe"] = xe_t
            state["xeT"] = xeT_t

            agg_phase(XT[:], g1idx_t, sel1_t, P.cw1, "g1", p1_win)
            if cc_mode == "single" and spmd:
                nc.gpsimd.collective_compute(
                    "AllGather", mybir.AluOpType.bypass,
                    replica_groups=[list(range(P.ncores))],
                    ins=[CCIN[:]], outs=[CCOUT[:]])
            agg_phase(CCOUT[:], g2idx_t, sel2_t, P.cw2, "g2", p2_win)

    nc.compile()
    return nc


# ------------------------------------------------------------------ runner ---
def make_in_maps(P, X, W, b, dt=DT):
    npdt = ml_dtypes.float8_e4m3 if dt == "fp8" else ml_dtypes.bfloat16
    KT = P.ch // 128
    xt = np.ascontiguousarray(X.astype(npdt))
    wt = np.ascontiguousarray(
        W.T.reshape(KT, 128, P.ch).transpose(1, 0, 2).astype(ml_dtypes.bfloat16))
    brep = np.ascontiguousarray(np.broadcast_to(b.astype(np.float32), (128, P.ch)))
    ident = np.eye(128, dtype=np.float32)
    in_maps = []
    for k in range(P.ncores):
        g1, s1 = P.p1[k]
        g2, s2 = P.p2[k]
        in_maps.append({
            "xt": xt, "wt": wt, "brep": brep, "ident": ident,
            "invd": P.invd[k],
            "g1idx": _wrap_idx(g1), "sel1": np.ascontiguousarray(s1.astype(npdt)),
            "g2idx": _wrap_idx(g2), "sel2": np.ascontiguousarray(s2.astype(npdt)),
        })
    return in_maps


def assemble(P, shards):
    out = np.zeros((P.nv, P.ch), np.float32)
    for k in range(P.ncores):
        vm = P.vmap[k]
        m = vm >= 0
        out[vm[m]] = shards[k][m]
    return out


_nc_cache = {}


def kernel(X, W, b, e2v_weight, v_idx, e_idx):
    global _last_results
    from concourse.bass_utils import run_bass_kernel_spmd

    P = make_plan(v_idx, e_idx, e2v_weight)
    key = (P.C1, P.C2, tuple(P.cw1), tuple(P.cw2), DT, CC_MODE)
    if key not in _nc_cache:
        _nc_cache[key] = build_nc(P)
    nc = _nc_cache[key]
    in_maps = make_in_maps(P, X, W, b)
    res = run_bass_kernel_spmd(nc, in_maps, list(range(P.ncores)), trace=TRACE)
    _last_results = res
    shards = [res.results[k]["out"] for k in range(P.ncores)]
    return assemble(P, shards)
